# revision 2
# baseline (speedup 1.0000x reference)
"""AttnPool Trainium2 kernel.

Math: the reference computes k = z @ W.T, scores = (q . k)/sqrt(D),
attn = softmax(scores over P), out = attn-weighted sum of z. Since q is a
single query vector, q.(z@W.T) == z.(q@W): precompute qw = q @ W (tiny, host),
then the device kernel is one memory-bound pass over z, pipelined at
[128, 768] row-tile granularity:

  DVE  scalar_tensor_tensor: s_t = sum(z_t * qw) * scale   (fused mul+reduce)
  ACT  exp:                  e_t = exp(s_t)                 (one column)
  PE   pooled accumulation:  acc += e_t.T @ z_t             (fp32r, PSUM)

then a tiny per-batch tail: S = sum(e) (PE ones-matmul + ACT accum),
rS = 1/S (DVE), and the output row is normalized during the PSUM->SBUF copy
(ACT, scale=rS) one batch late so no engine stream stalls across batches.

z is streamed from HBM exactly once: 24 MiB/core -> ~70 us at ~358 GB/s per
core; all compute hides under the DMA stream.

Sharding: data-parallel over batch, 8 batches per core on 8 cores (SPMD).
"""
import os

os.environ.setdefault("NEURON_RT_RESET_CORES", "1")

import numpy as np

import concourse.tile as tile
from concourse import bacc, mybir
from concourse.bass_utils import run_bass_kernel_spmd

B, P, D = 64, 1024, 768
N_CORES = 8
B_PER_CORE = B // N_CORES
P_TILES = P // 128
SCALE = float(1.0 / np.sqrt(np.float32(D)))
HALF = D // 2

f32 = mybir.dt.float32
f32r = mybir.dt.float32r

_cache = {}


def make_pools(tc):
    return (
        tc.tile_pool(name="consts", bufs=1),
        tc.tile_pool(name="zp", bufs=3),
        tc.tile_pool(name="sc", bufs=2),
        tc.tile_pool(name="scr", bufs=3),
        tc.tile_pool(name="ps", bufs=2, space="PSUM"),
    )


def emit_consts(nc, consts, qw_dram):
    # qw broadcast rides SWDGE so it never delays the z stream (HWDGE)
    qw_bc = consts.tile([128, D], f32, name="qw_bc")
    nc.gpsimd.dma_start(out=qw_bc[:], in_=qw_dram.to_broadcast((128, D)))
    ones_col = consts.tile([128, 1], f32, name="ones_col")
    nc.vector.memset(ones_col[:], 1.0)
    junk_row = consts.tile([1, P_TILES], f32, name="junk_row")
    return qw_bc, ones_col, junk_row


def emit_body(nc, tc, pools, consts, z_dram, out_dram, dma_tiles=2):
    """One full pass over this core's 8 batches, incl. all 8 output rows."""
    _, zp, scp, scrp, psp = pools
    qw_bc, ones_col, junk_row = consts

    def emit_out(prev):
        b_prev, pool_prev0, pool_prev1, S_prev = prev
        rS = scp.tile([1, 1], f32, name="rS", tag="rS")
        nc.vector.reciprocal(rS[:], S_prev[0:1, 0:1])
        out_row = scp.tile([1, D], f32, name="out_row", tag="out_row")
        for h, pps in enumerate([pool_prev0, pool_prev1]):
            nc.scalar.activation(
                out=out_row[0:1, h * HALF : (h + 1) * HALF],
                in_=pps[:],
                func=mybir.ActivationFunctionType.Copy,
                scale=rS[0:1, 0:1],
            )
        nc.scalar.dma_start(out=out_dram[b_prev : b_prev + 1, :], in_=out_row[:])

    prev = None
    for b in range(B_PER_CORE):
        z_sb = zp.tile([128, P_TILES, D], f32r, name="z_sb", tag="z_sb")
        s_buf = scp.tile([128, P_TILES], f32, name="s_buf", tag="s_buf")
        e_buf = scp.tile([128, P_TILES], f32r, name="e_buf", tag="e_buf")
        pool_ps0 = psp.tile([1, HALF], f32, name="pool_ps0", tag="pool_ps0")
        pool_ps1 = psp.tile([1, HALF], f32, name="pool_ps1", tag="pool_ps1")

        for t in range(P_TILES):
            if t % dma_tiles == 0:
                nc.sync.dma_start(
                    out=z_sb[:, t : t + dma_tiles, :],
                    in_=z_dram[b, t * 128 : (t + dma_tiles) * 128, :]
                    .rearrange("(g p) d -> p g d", p=128)
                    .bitcast(f32r),
                )
            scratch = scrp.tile([128, D], f32, name="scratch", tag="scratch")
            nc.vector.scalar_tensor_tensor(
                out=scratch[:],
                in0=z_sb[:, t, :].bitcast(f32),
                scalar=SCALE,
                in1=qw_bc[:],
                op0=mybir.AluOpType.mult,
                op1=mybir.AluOpType.mult,
                accum_out=s_buf[:, t : t + 1],
            )
            nc.scalar.activation(
                out=e_buf[:, t : t + 1],
                in_=s_buf[:, t : t + 1],
                func=mybir.ActivationFunctionType.Exp,
            )
            for h, pps in enumerate([pool_ps0, pool_ps1]):
                nc.tensor.matmul(
                    out=pps[:],
                    lhsT=e_buf[:, t : t + 1],
                    rhs=z_sb[:, t, h * HALF : (h + 1) * HALF],
                    start=(t == 0),
                    stop=(t == P_TILES - 1),
                )

        # softmax denominator: S = sum over all 1024 e values
        S_row = psp.tile([1, P_TILES], f32, name="S_row", tag="S_row")
        nc.tensor.matmul(
            out=S_row[:], lhsT=ones_col[:], rhs=e_buf[:].bitcast(f32), start=True, stop=True
        )
        S_val = scp.tile([1, 1], f32, name="S_val", tag="S_val")
        nc.scalar.activation(
            out=junk_row[:],
            in_=S_row[:],
            func=mybir.ActivationFunctionType.Copy,
            accum_out=S_val[:],
        )

        if prev is not None:
            emit_out(prev)
        prev = (b, pool_ps0, pool_ps1, S_val)

    emit_out(prev)


def build(reps=1, dma_tiles=2):
    nc = bacc.Bacc("TRN2", target_bir_lowering=False, debug=False, num_devices=N_CORES)
    z_dram = nc.dram_tensor("z", [B_PER_CORE, P, D], f32, kind="ExternalInput").ap()
    qw_dram = nc.dram_tensor("qw", [1, D], f32, kind="ExternalInput").ap()
    out_dram = nc.dram_tensor("out", [B_PER_CORE, D], f32, kind="ExternalOutput").ap()

    with tile.TileContext(nc) as tc:
        pools_cm = make_pools(tc)
        with (
            pools_cm[0] as consts,
            pools_cm[1] as zp,
            pools_cm[2] as scp,
            pools_cm[3] as scrp,
            pools_cm[4] as psp,
        ):
            pools = (consts, zp, scp, scrp, psp)
            ck = emit_consts(nc, consts, qw_dram)
            for rep in range(reps):
                emit_body(nc, tc, pools, ck, z_dram, out_dram, dma_tiles=dma_tiles)

    nc.finalize()
    return nc


def get_nc(reps=1, dma_tiles=2):
    key = (reps, dma_tiles)
    if key not in _cache:
        _cache[key] = build(reps, dma_tiles)
    return _cache[key]


def run(z, qw, reps=1, **kwargs):
    """Run the SPMD kernel. z: [B,P,D] f32, qw: [D] f32. Returns results obj."""
    nc = get_nc(reps)
    in_maps = [
        {"z": z[i * B_PER_CORE : (i + 1) * B_PER_CORE], "qw": qw[None, :]}
        for i in range(N_CORES)
    ]
    return run_bass_kernel_spmd(nc, in_maps, core_ids=list(range(N_CORES)), **kwargs)


def kernel(z, q, W_proj):
    z = np.ascontiguousarray(np.asarray(z, dtype=np.float32))
    q = np.asarray(q, dtype=np.float32)
    W_proj = np.asarray(W_proj, dtype=np.float32)
    qw = (q.reshape(D) @ W_proj).astype(np.float32)

    res = run(z, qw)
    out = np.concatenate([r["out"] for r in res.results], axis=0)
    return out.astype(np.float32)


if __name__ == "__main__":
    rng = np.random.default_rng(0)
    z = rng.standard_normal((B, P, D)).astype(np.float32)
    q = rng.standard_normal((1, 1, D)).astype(np.float32)
    W = (rng.standard_normal((D, D)) / np.sqrt(D)).astype(np.float32)
    out = kernel(z, q, W)
    print("out", out.shape, out.dtype, out[:2, :4])

